# revision 1
# baseline (speedup 1.0000x reference)
"""Trainium2 Bass kernel for nn_MultiHeadSelfAttention_90537910600353.

Reference computation (B=2, S=2048, E=1024, H=16, d=64):
    L_h       = tril(params[h])                      # [64,64] per head
    Wqk_h     = L_h @ L_h^T                          # PSD
    scores    = xh Wqk xh^T / sqrt(d)  = (xh Lh/8^.5)(xh Lh/8^.5)^T
    attn      = softmax(scores)
    V         = x @ Wv^T + bv
    out       = (attn @ Vh) @ Wo^T + bo

Sharding: batch*head parallel over 8 cores. Core m handles b = m//4 and
heads [4*(m%4), 4*(m%4)+4). Each core emits a partial [S, E] = its heads'
(attn@Vh) @ Wo_h^T contribution; host sums the 4 partials per batch + bo.

Device algorithm per core (all matmuls fp32r ~= tf32 inputs + fp32 accum):
  z~_h  = xh_h @ (L_h / 8^0.5)            stored transposed [64, S]
  M_h   = diag scores + 24 = |z~_q|^2 + 24  (softmax max-bound; margin keeps
          exp args < ~60 since max off-diag excess over diag is ~80 for this
          distribution, and exp(arg) stays in fp32 range either way)
  E^T   = exp(z~^T z~ - M[q])  tiles [t,128 x q,512] via K=64 matmul plus a
          concurrent K=1 aug-row matmul (ones x -M) from partition 64
  V'    = [x @ Wv_h^T + bv_h | 1] per head  -> [t, 65] tiles (aug ones col
          makes attn@V also produce the softmax denominator)
  out'^T= V'^T @ E^T   accumulated over t   -> [65, q] (row 64 = denom)
  outn  = out'[0:64] * bcast(1/denom)  (bcast via K=1 ones matmul)
  partial = outn^T_heads @ Wo_heads^T       -> [q, E] -> DRAM

Self-contained: only needs numpy + the concourse stack at /opt/trn_rl_repo.
"""

import sys

if "/opt/trn_rl_repo" not in sys.path:
    sys.path.insert(0, "/opt/trn_rl_repo")

import numpy as np

import concourse.bass as bass
import concourse.mybir as mybir
import concourse.tile as tile
from concourse.bass_utils import run_bass_kernel_spmd

F32 = mybir.dt.float32
F32R = mybir.dt.float32r

B, S, E, H = 2, 2048, 1024, 16
D = 64
NCORES = 8
HPC = 4            # heads per core
MARGIN = 24.0
NT = S // 128      # 16 t blocks
NQ = S // 512      # 4 q chunks of 512


def _split_multi_waits(nc):
    """This walrus build rejects instructions carrying more than one sync
    wait. Hoist extra waits onto same-engine NOPs inserted just before."""
    for f in nc.m.functions:
        for b in f.blocks:
            il = b.instructions
            i = 0
            while i < len(il):
                inst = il[i]
                si = getattr(inst, "sync_info", None)
                if si is not None and si.on_wait and len(si.on_wait) > 1:
                    waits = list(si.on_wait)
                    for w in waits[:-1]:
                        nop = mybir.InstNoOp(
                            name=nc.get_next_instruction_name(),
                            engine=inst.engine,
                            sync_info=mybir.SyncInfo(on_wait=[w], on_update=[]),
                        )
                        il.insert(i, nop)
                        i += 1
                    inst.sync_info = mybir.SyncInfo(
                        on_wait=[waits[-1]], on_update=si.on_update
                    )
                i += 1


def _tf32(v):
    """RNE-round fp32 -> tf32 bit pattern (what the PE reads for fp32r)."""
    vb = np.ascontiguousarray(v, np.float32).view(np.uint32)
    return ((vb + 0x1000) & 0xFFFFE000).view(np.float32).copy()


def build_program(repeat=1, loop_n=0):
    nc = bass.Bass("TRN2", target_bir_lowering=False, debug=False, num_devices=NCORES)

    xt = nc.dram_tensor("xt", [E, S], F32R, kind="ExternalInput").ap()
    lp = nc.dram_tensor("lp", [2, 128, 128], F32R, kind="ExternalInput").ap()
    wv = nc.dram_tensor("wv", [E + 1, HPC * 65], F32R, kind="ExternalInput").ap()
    wo = nc.dram_tensor("wo", [HPC * D, E], F32R, kind="ExternalInput").ap()
    hsel = nc.dram_tensor("hsel", [128, 33], F32R, kind="ExternalInput").ap()
    onesz = nc.dram_tensor("onesz", [1, HPC, S], F32R, kind="ExternalInput").ap()
    onesrow = nc.dram_tensor("onesrow", [1, 128], F32R, kind="ExternalInput").ap()
    partial = nc.dram_tensor("partial", [S, E], F32, kind="ExternalOutput").ap()

    with tile.TileContext(nc) as tc:
      import contextlib
      for _rep in range(repeat):
       with (tc.For_i(0, loop_n, 1) if loop_n else contextlib.nullcontext()):
        with (
            tc.tile_pool(name="consts", bufs=1) as consts,
            tc.tile_pool(name="zsb", bufs=1) as zsb,
            tc.tile_pool(name="sqp", bufs=2) as sqp,
            tc.tile_pool(name="vsb", bufs=1) as vsb,
            tc.tile_pool(name="onsb", bufs=1) as onsb,
        ):
            ones_row = consts.tile([1, 128], F32R)
            nc.gpsimd.dma_start(out=ones_row[:], in_=onesrow[:])
            hsel_t = consts.tile([128, 33], F32R)
            nc.gpsimd.dma_start(out=hsel_t[:], in_=hsel[:])
            lp_t = consts.tile([128, 2, 128], F32R)
            for j in range(2):
                nc.gpsimd.dma_start(out=lp_t[:, j, :], in_=lp[j, :, :])
            wv_t = consts.tile([128, 8, HPC * 65], F32R)
            for k in range(8):
                nc.sync.dma_start(out=wv_t[:, k, :], in_=wv[128 * k : 128 * (k + 1), :])
            wv_bias = consts.tile([1, HPC * 65], F32R)
            nc.gpsimd.dma_start(out=wv_bias[:], in_=wv[E : E + 1, :])

            # zA[0:64, h, :] = z~_h^T ; zA[64, h, :] = -(|z~_q|^2) - MARGIN
            zA = zsb.tile([65, HPC, S], F32R)
            # zOnes: same z rows, row 64 = 1.0 (lhsT side of the K=65 S-matmul)
            zOnes = zsb.tile([65, HPC, S], F32R)
            nc.gpsimd.dma_start(out=zOnes[64:65, :, :], in_=onesz[:])

            with (
                tc.tile_pool(name="xtp", bufs=1) as xtp,
                tc.tile_pool(name="zp", bufs=2, space="PSUM") as zp,
                tc.tile_pool(name="mp", bufs=2, space="PSUM") as mp,
            ):
                xt_t = xtp.tile([128, 8, S], F32R)
                for k in range(8):
                    nc.sync.dma_start(out=xt_t[:, k, :], in_=xt[128 * k : 128 * (k + 1), :])
                for j in range(2):  # head pair = xt chunk j
                    for qc in range(NQ):
                        ql = slice(512 * qc, 512 * (qc + 1))
                        zp_t = zp.tile([128, 512], F32)
                        nc.tensor.matmul(
                            zp_t[:], lp_t[:, j, :], xt_t[:, j, ql], start=True, stop=True
                        )
                        sq_t = sqp.tile([128, 512], F32R)
                        nc.scalar.activation(
                            sq_t[:], zp_t[:], mybir.ActivationFunctionType.Square
                        )
                        mp_t = mp.tile([33, 512], F32)
                        nc.tensor.matmul(mp_t[:], hsel_t[:], sq_t[:], start=True, stop=True)
                        for i in range(2):
                            h = 2 * j + i
                            nc.vector.tensor_copy(
                                zA[0:64, h, ql], zp_t[64 * i : 64 * i + 64, :]
                            )
                            nc.gpsimd.dma_start(
                                out=zOnes[0:64, h, ql], in_=zA[0:64, h, ql]
                            )
                            nc.vector.tensor_scalar(
                                zA[64:65, h, ql],
                                mp_t[32 * i : 32 * i + 1, :],
                                -1.0,
                                -MARGIN,
                                mybir.AluOpType.mult,
                                mybir.AluOpType.add,
                            )

                # V' tiles: v_all[:, t, 65h:65h+64] = V_h rows, col 65h+64 = 1
                v_all = vsb.tile([128, NT, HPC * 65], F32R)
                with tc.tile_pool(name="vp", bufs=2, space="PSUM") as vp:
                    for t in range(NT):
                        tl = slice(128 * t, 128 * (t + 1))
                        vp_t = vp.tile([128, HPC * 65], F32)
                        for k in range(8):
                            nc.tensor.matmul(
                                vp_t[:],
                                xt_t[:, k, tl],
                                wv_t[:, k, :],
                                start=(k == 0),
                                stop=False,
                            )
                        nc.tensor.matmul(
                            vp_t[:], ones_row[0:1, :], wv_bias[:], start=False, stop=True
                        )
                        nc.vector.tensor_copy(v_all[:, t, :], vp_t[:])

            # outn^T, heads packed in pairs: on_t[64*(h%2):..., h//2, :]
            on_t = onsb.tile([128, 2, S], F32R)

            with (
                tc.tile_pool(name="sp", bufs=2, space="PSUM") as sp,
                tc.tile_pool(name="op", bufs=2, space="PSUM") as op,
                tc.tile_pool(name="ep", bufs=8) as ep,
                tc.tile_pool(name="nsb", bufs=2) as nsb,
            ):
                QH = S // 2  # 1024-wide q halves: sp/op/eT fit 8 PSUM banks

                def normalize(h, qh, op_t):
                    # out'[0:64] / denom (row 64); bc reuses the freed op slot
                    rc_t = nsb.tile([1, QH], F32R, tag="rc")
                    with nc.allow_low_precision(reason="f32r recip feeds matmul"):
                        nc.vector.reciprocal(rc_t[:], op_t[64:65, :])
                    ot_t = nsb.tile([64, QH], F32, tag="ot")
                    nc.vector.tensor_copy(ot_t[:], op_t[0:64, :])
                    bc_t = op.tile([64, QH], F32, tag="op")
                    for qc in range(2):
                        ql = slice(512 * qc, 512 * (qc + 1))
                        nc.tensor.matmul(
                            bc_t[:, ql],
                            ones_row[0:1, 0:64],
                            rc_t[0:1, ql],
                            start=True,
                            stop=True,
                        )
                    nc.vector.tensor_mul(
                        on_t[
                            64 * (h % 2) : 64 * (h % 2) + 64,
                            h // 2,
                            QH * qh : QH * (qh + 1),
                        ],
                        ot_t[:],
                        bc_t[:],
                    )

                pending = None
                first = [True]
                for h in range(HPC):
                    for qh in range(2):
                        op_t = op.tile([65, QH], F32, tag="op")
                        if first[0]:
                            first[0] = False
                            pre_sp, pre_eT = [], []
                            for t in range(NT):
                                tl = slice(128 * t, 128 * (t + 1))
                                sp_t = sp.tile([128, QH], F32, tag="sp")
                                for qc in range(2):
                                    qg = slice(512 * qc, 512 * (qc + 1))
                                    ql = slice(512 * qc, 512 * (qc + 1))
                                    nc.tensor.matmul(
                                        sp_t[:, ql], zOnes[0:65, 0, tl],
                                        zA[0:65, 0, qg], start=True, stop=True)
                                eT_t = ep.tile([128, QH], F32R, tag="eT")
                                nc.scalar.activation(eT_t[:], sp_t[:], mybir.ActivationFunctionType.Exp)
                                pre_eT.append(eT_t)
                            for t in range(NT):
                                for qc in range(2):
                                    ql = slice(512 * qc, 512 * (qc + 1))
                                    nc.tensor.matmul(
                                        op_t[:, ql], v_all[:, t, 0:65],
                                        pre_eT[t][:, ql], start=(t == 0), stop=(t == NT - 1))
                            pending = (0, 0, op_t)
                            continue
                        for t in range(NT):
                            tl = slice(128 * t, 128 * (t + 1))
                            sp_t = sp.tile([128, QH], F32, tag="sp")
                            for qc in range(2):
                                qg = slice(
                                    QH * qh + 512 * qc, QH * qh + 512 * (qc + 1)
                                )
                                ql = slice(512 * qc, 512 * (qc + 1))
                                nc.tensor.matmul(
                                    sp_t[:, ql],
                                    zOnes[0:65, h, tl],
                                    zA[0:65, h, qg],
                                    start=True,
                                    stop=True,
                                )
                            eT_t = ep.tile([128, QH], F32R, tag="eT")
                            nc.scalar.activation(
                                eT_t[:], sp_t[:], mybir.ActivationFunctionType.Exp
                            )
                            for qc in range(2):
                                ql = slice(512 * qc, 512 * (qc + 1))
                                nc.tensor.matmul(
                                    op_t[:, ql],
                                    v_all[:, t, 65 * h : 65 * h + 65],
                                    eT_t[:, ql],
                                    start=(t == 0),
                                    stop=(t == NT - 1),
                                )
                            if t == 6 and pending is not None:
                                normalize(*pending)
                                pending = None
                        pending = (h, qh, op_t)
                normalize(*pending)


            # partial[q, :] = outn^T.T @ Wo^T (contraction over this core's d)
            wo_t = consts.tile([128, 2, E], F32R)
            for c in range(2):
                nc.sync.dma_start(
                    out=wo_t[:, c, :], in_=wo[128 * c : 128 * (c + 1), :]
                )
            with (
                tc.tile_pool(name="wop", bufs=2, space="PSUM") as wop,
                tc.tile_pool(name="wsb", bufs=3) as wsb,
            ):
                for qb in range(NT):
                    qbl = slice(128 * qb, 128 * (qb + 1))
                    wop_t = wop.tile([128, 512], F32, tag="wop")
                    wop_t2 = wop.tile([128, 512], F32, tag="wop2")
                    for c in range(2):
                        nc.tensor.matmul(
                            wop_t[:],
                            on_t[:, c, qbl],
                            wo_t[:, c, 0:512],
                            start=(c == 0),
                            stop=(c == 1),
                        )
                        nc.tensor.matmul(
                            wop_t2[:],
                            on_t[:, c, qbl],
                            wo_t[:, c, 512:1024],
                            start=(c == 0),
                            stop=(c == 1),
                        )
                    ws_t = wsb.tile([128, E], F32)
                    nc.vector.tensor_copy(ws_t[:, 0:512], wop_t[:])
                    nc.scalar.copy(ws_t[:, 512:1024], wop_t2[:])
                    nc.sync.dma_start(out=partial[qbl, :], in_=ws_t[:])

    _split_multi_waits(nc)
    return nc


_prog_cache = {}


def _get_program():
    if "nc" not in _prog_cache:
        _prog_cache["nc"] = build_program()
    return _prog_cache["nc"]


def make_in_maps(x, params, Wv, bv, Wo, bo):
    """Host-side sharding/layout prep. Returns per-core input dicts."""
    x = np.asarray(x, np.float32)
    params = np.asarray(params, np.float32)
    Wv = np.asarray(Wv, np.float32)
    bv = np.asarray(bv, np.float32)
    Wo = np.asarray(Wo, np.float32)

    rows, cols = np.tril_indices(D)
    L = np.zeros((H, D, D), np.float32)
    L[:, rows, cols] = params
    Ls = L / np.float32(np.sqrt(8.0))

    hsel = np.zeros((128, 33), np.float32)
    hsel[0:64, 0] = 1.0
    hsel[64:128, 32] = 1.0
    onesz = np.ones((1, HPC, S), np.float32)
    onesrow = np.ones((1, 128), np.float32)

    xT = [np.ascontiguousarray(x[b].T) for b in range(B)]

    in_maps = []
    for m in range(NCORES):
        b = m // 4
        hbase = HPC * (m % 4)
        heads = list(range(hbase, hbase + HPC))
        # xt rows reordered: this core's 4 head chunks first (256 rows),
        # then the remaining 768 rows (order irrelevant for V contraction).
        own = list(range(hbase * D, (hbase + HPC) * D))
        rest = [e for e in range(E) if not (hbase * D <= e < (hbase + HPC) * D)]
        perm = own + rest
        xt_m = _tf32(xT[b][perm, :])

        lp_m = np.zeros((2, 128, 128), np.float32)
        for j in range(2):
            lp_m[j, 0:64, 0:64] = Ls[heads[2 * j]]
            lp_m[j, 64:128, 64:128] = Ls[heads[2 * j + 1]]
        lp_m = _tf32(lp_m)

        # wv: [E+1, 260]; col block h: [Wv_head^T | 0], bias row: [bv | 1],
        # e-rows permuted to match xt row order.
        wv_m = np.zeros((E + 1, HPC * 65), np.float32)
        for i, h in enumerate(heads):
            wv_m[0:E, 65 * i : 65 * i + 64] = Wv[h * D : (h + 1) * D, perm].T
            wv_m[E, 65 * i : 65 * i + 64] = bv[h * D : (h + 1) * D]
            wv_m[E, 65 * i + 64] = 1.0
        wv_m = _tf32(wv_m)

        wo_m = _tf32(
            np.ascontiguousarray(Wo[:, hbase * D : (hbase + HPC) * D].T)
        )

        in_maps.append(
            {
                "xt": xt_m,
                "lp": lp_m,
                "wv": wv_m,
                "wo": wo_m,
                "hsel": hsel,
                "onesz": onesz,
                "onesrow": onesrow,
            }
        )
    return in_maps


def run(x, params, Wv, bv, Wo, bo, trace=False):
    nc = _get_program()
    in_maps = make_in_maps(x, params, Wv, bv, Wo, bo)
    r = run_bass_kernel_spmd(nc, in_maps, list(range(NCORES)), trace=trace)
    bo = np.asarray(bo, np.float32)
    out = np.zeros((B, S, E), np.float32)
    for b in range(B):
        acc = np.zeros((S, E), np.float64)
        for m in range(4 * b, 4 * b + 4):
            acc += r.results[m]["partial"].astype(np.float64)
        out[b] = (acc + bo).astype(np.float32)
    return out, r


def kernel(x, params, Wv, bv, Wo, bo):
    out, _ = run(x, params, Wv, bv, Wo, bo, trace=False)
    return out

